# revision 48
# baseline (speedup 1.0000x reference)
"""CPCC loss (1 - Pearson(tree_d, proto_d)) on 8 Trainium2 NeuronCores.

Strategy (data-parallel, per sharding hint), v2 "sorted fp8":
  - Shard representations/target_fine along N across the 8 cores (contiguous
    32768-row blocks).
  - Host-side per core: sort tokens by fine class, cast reps to fp8_e4m3
    (empirical loss error vs f32 reference: ~1.6e-4 — the segment means
    average the quantization noise out), lay out as [128, 256 chunks, 128]
    so each DMA tile is 4 KiB-contiguous per partition (line-rate
    descriptors, plain HWDGE copy, no cast).
  - Because tokens are class-sorted, each 128-token chunk spans only ~2
    adjacent classes. The per-chunk one-hot collapses to a [128, W=8] fp8
    sliver (precomputed on host, 262 KiB total) whose psum column window
    off_c = clip(100*c/256 - 3) is STATIC for uniform targets (boundary
    drift is ~0.3 classes, window slack is +-3); host verifies and falls
    back to data-derived offsets (program rebuild) for adversarial inputs.
  - Device stream: per chunk one PE matmul  acc[:, off:off+W] +=
    rt_chunk.T @ oh_sliver  with the fp8 reps as the FWL fast-load
    stationary operand and the W=8-column sliver as the moving operand
    (~60-cycle floor per matmul). No DVE work in the stream phase at all.
    PSUM accumulates S.T = [128 dims, 100 classes] in f32.
  - AllReduce(add) the [128, 100] f32 partials, then every core runs the
    replicated tail, restructured for minimal serial-chain latency (the
    HW pays ~0.5-1us per cross-engine handoff, so op/handoff count rules):
    transpose S.T -> S -> P = S*(1/counts) (host counts); XT = [P.T |
    P.T@wm | P.T@(wm@wc)] built by three PE ops into one psum bank (the
    wm@wc fold skips materializing M); per-level Grams from XT free-dim
    slices; the three dist sqrts run on ACT STRAIGHT FROM PSUM as
    sqrt(2*psum + 1e-6) with a negated Gram operand so psum holds +d^2/2
    (no DVE clamp ops); the 1e-6 bias replaces the max(.,0) clamp and its
    known diagonal contribution to the Pearson sums is removed exactly by
    a constant 101st accumulator row folded into the PE reduce.
    Pearson sums via DVE accum_out, then a short scalar finale.

Precision: reps are rounded to fp8_e4m3 (matmul operand); the one-hot
sliver is exact (0/1 in fp8), accumulation is f32 in PSUM and the whole
tail is f32. Observed loss error vs the f32 reference is ~5.6e-4
(gate 2e-2): ~1.6e-4 from fp8 and ~4e-4 from the sqrt-bias warp of the
off-diagonal distances.
"""

import numpy as np

C_FINE, C_MID, C_COARSE = 100, 20, 5
C_ALL = C_FINE + C_MID + C_COARSE    # 125
EPS = 1e-12
N_CORES = 8
N, D = 262144, 128
N_LOC = N // N_CORES            # 32768 rows per core
CHUNK = 128                     # contraction size per matmul
N_CHUNKS = N_LOC // CHUNK       # 256
W = 8                           # one-hot sliver width (classes per window)
NPAIRS = C_FINE * (C_FINE - 1) // 2   # 4950
DB = 1e-6   # dist sqrt bias (clamp replacement; diag corrected exactly)

# DMA tiles (in chunks): small first tile to cut time-to-first-matmul;
# 64-chunk bodies give 8 KiB-contiguous descriptors (HW-measured fastest)
TILES = [(0, 8), (8, 56)] + [(64 * i, 64) for i in range(1, 4)]

_CACHE = {}


def _static_offsets():
    """Window start per chunk; static for uniform targets."""
    return tuple(
        int(np.clip((C_FINE * c) // N_CHUNKS - 3, 0, C_FINE - W))
        for c in range(N_CHUNKS))


def _build_program(offsets=None, stream_reps=1, loop_reps=1, dma_only=False,
                   no_cc=False, cc_tail_reps=1, cc_kind="AllReduce",
                   chain_cc=True, tiles=None, alt_rings=False, debug=False):
    """Build the SPMD program.

    Benchmarking knobs (the graded kernel uses all defaults):
      stream_reps>1 statically unrolls the streaming phase (same data).
      loop_reps>1 wraps the streaming phase in a dynamic For_i loop (slope
        timing); psum restarts each rep so the output stays correct.
      dma_only=True keeps only 1 matmul per tile.
      no_cc=True builds a single-core program without the AllReduce (for
        TimelineSim cost-model analysis).
      cc_tail_reps>1 serially chains the AllReduce+tail section that many
        times (slope timing of the non-streaming part; output garbage).
    """
    import contextlib

    import concourse.bacc as bacc
    import concourse.mybir as mybir
    import concourse.tile as tile
    from concourse.bass import MemorySpace
    from concourse.tile import add_dep_helper

    if offsets is None:
        offsets = _static_offsets()
    if tiles is None:
        tiles = TILES

    f32 = mybir.dt.float32
    bf16 = mybir.dt.bfloat16
    fp8 = mybir.dt.float8e4
    Alu = mybir.AluOpType
    Act = mybir.ActivationFunctionType
    X = mybir.AxisListType.X

    nc = bacc.Bacc("TRN2", target_bir_lowering=False, debug=False,
                   num_devices=1 if no_cc else N_CORES)

    reps_d = nc.dram_tensor("reps", [128, N_CHUNKS * D], fp8,
                            kind="ExternalInput")
    ohs_d = nc.dram_tensor("ohs", [128, N_CHUNKS * W], fp8,
                           kind="ExternalInput")
    rec_d = nc.dram_tensor("rec", [C_FINE, 1], f32, kind="ExternalInput")
    ident_d = nc.dram_tensor("ident", [128, 128], f32, kind="ExternalInput")
    ones_d = nc.dram_tensor("ones", [128, 128], f32, kind="ExternalInput")
    emt_d = nc.dram_tensor("emt", [C_MID, C_FINE], f32, kind="ExternalInput")
    ect_d = nc.dram_tensor("ect", [C_COARSE, C_FINE], f32,
                           kind="ExternalInput")
    wm_d = nc.dram_tensor("wm", [C_FINE, C_MID], f32, kind="ExternalInput")
    wmc_d = nc.dram_tensor("wmc", [C_FINE, C_COARSE], f32,
                           kind="ExternalInput")
    fcor_d = nc.dram_tensor("fcor", [1, 5], f32, kind="ExternalInput")
    onesb_d = nc.dram_tensor("onesb", [C_FINE + 1, 1], f32,
                             kind="ExternalInput")
    loss_d = nc.dram_tensor("loss", [1, 1], f32, kind="ExternalOutput")
    if debug:
        dbg_d = nc.dram_tensor("dbg", [128, 256], f32, kind="ExternalOutput")

    with tile.TileContext(nc) as tc:
        with (
            tc.tile_pool(name="const", bufs=1) as cpool,
            tc.tile_pool(name="reps", bufs=4) as rpool,
            tc.tile_pool(name="work", bufs=1) as wpool,
            tc.tile_pool(name="acc", bufs=1, space=MemorySpace.PSUM) as apool,
            tc.tile_pool(name="tps", bufs=6, space=MemorySpace.PSUM) as ppool,
            tc.tile_pool(name="dram", bufs=1, space=MemorySpace.DRAM) as dpool,
        ):
            # ---- constants (one-hot slivers first: they gate the matmuls;
            # issued on the scalar HWDGE ring so the reps DMAs on the sync
            # ring aren't queued behind them) ----
            ohs_t = cpool.tile([128, N_CHUNKS, W], fp8)
            nc.scalar.dma_start(
                ohs_t[:], ohs_d[:].rearrange("p (c w) -> p c w", w=W))
            ident_t = cpool.tile([128, 128], f32)
            nc.scalar.dma_start(ident_t[:], ident_d[:])
            ones_t = cpool.tile([128, 128], f32)
            nc.scalar.dma_start(ones_t[:], ones_d[:])
            rec_t = cpool.tile([C_FINE, 1], f32)
            nc.scalar.dma_start(rec_t[:], rec_d[:])
            emt_t = cpool.tile([C_MID, C_FINE], f32)
            nc.scalar.dma_start(emt_t[:], emt_d[:])
            ect_t = cpool.tile([C_COARSE, C_FINE], f32)
            nc.scalar.dma_start(ect_t[:], ect_d[:])
            wm_t = cpool.tile([C_FINE, C_MID], f32)
            nc.scalar.dma_start(wm_t[:], wm_d[:])
            wmc_t = cpool.tile([C_FINE, C_COARSE], f32)
            nc.scalar.dma_start(wmc_t[:], wmc_d[:])
            onesb_t = cpool.tile([C_FINE + 1, 1], f32)
            nc.scalar.dma_start(onesb_t[:], onesb_d[:])
            identb_t = cpool.tile([128, 128], bf16)
            nc.vector.tensor_copy(identb_t[:], ident_t[:])
            # Pearson accumulator; row C_FINE holds the constant exact
            # correction for the sqrt-bias diagonal terms (loaded once)
            red = wpool.tile([C_FINE + 1, 8], f32, tag="red")
            nc.scalar.dma_start(red[C_FINE:C_FINE + 1, 0:5], fcor_d[:])
            db_t = cpool.tile([128, 1], f32)
            nc.vector.memset(db_t[:], DB)
            eps_t = cpool.tile([128, 1], f32)
            nc.vector.memset(eps_t[:], EPS)

            # ---- main streaming loop: S.T partial sums in psum ----
            acc = apool.tile([128, C_FINE], f32)
            loop_cm = (tc.For_i(0, loop_reps, 1) if loop_reps > 1
                       else contextlib.nullcontext())
            with loop_cm:
                for rep in range(stream_reps):
                    for ti, (c0, ntc) in enumerate(tiles):
                        rt = rpool.tile([128, ntc, D], fp8,
                                        tag=f"rt{ntc}")
                        src = reps_d[:, c0 * D:(c0 + ntc) * D]
                        eng = (nc.scalar if (alt_rings and ti % 2)
                               else nc.sync)
                        eng.dma_start(
                            rt[:], src.rearrange("p (k d) -> p k d", d=D))
                        ks = [0] if dma_only else list(range(ntc))
                        for k in ks:
                            c = c0 + k
                            off = offsets[c]
                            nc.tensor.matmul(
                                acc[:, off:off + W], rt[:, k, :],
                                ohs_t[:, c, :],
                                start=(rep == 0 and ti == 0 and k == ks[0]),
                                stop=(rep == stream_reps - 1
                                      and ti == len(tiles) - 1
                                      and k == ks[-1]),
                            )

            part_t = wpool.tile([128, C_FINE], bf16)
            nc.vector.tensor_copy(part_t[:], acc[:])

            if not no_cc:
                if cc_kind == "AllToAll":
                    # replicated-input AllToAll == one-phase direct AllGather
                    cc_in = dpool.tile([N_CORES * 128, C_FINE], bf16)
                    cc_out = dpool.tile([N_CORES * 128, C_FINE], bf16)
                elif cc_kind == "AllReduce":
                    cc_in = dpool.tile([128, C_FINE], bf16)
                    cc_out = dpool.tile([128, C_FINE], bf16)
                else:
                    cc_in = dpool.tile([128, C_FINE], bf16)
                    cc_out = dpool.tile([N_CORES * 128, C_FINE], bf16)

            def all_summed(after=None):
                """AllReduce (or AllGather/AllToAll + local sum).

                `after`: BassInstruction to serialize behind (bench chains:
                a pure dependency edge replaces the old data-poke op).
                """
                if no_cc:
                    return part_t
                if cc_kind == "AllToAll":
                    dm = nc.gpsimd.dma_start(
                        cc_in[:].rearrange("(r p) f -> p r f", r=N_CORES),
                        part_t[:].rearrange("p (r f) -> p r f", r=1)
                        .broadcast_to([128, N_CORES, C_FINE]))
                else:
                    dm = nc.sync.dma_start(cc_in[:], part_t[:])
                if after is not None:
                    add_dep_helper(dm.ins, after.ins, sync=True,
                                   reason="serial cc_tail chain (bench)")
                nc.gpsimd.collective_compute(
                    cc_kind,
                    (mybir.AluOpType.add if cc_kind == "AllReduce"
                     else mybir.AluOpType.bypass),
                    replica_groups=[list(range(N_CORES))],
                    ins=[cc_in.opt()],
                    outs=[cc_out.opt()],
                )
                if cc_kind == "AllReduce":
                    ST = wpool.tile([128, C_FINE], bf16)
                    nc.sync.dma_start(ST[:], cc_out[:])
                    return ST
                gath = wpool.tile([128, N_CORES, C_FINE], f32)
                nc.sync.dma_start(
                    gath[:],
                    cc_out[:].rearrange("(r p) f -> p r f", r=N_CORES))
                nc.vector.tensor_add(gath[:, 0:4, :], gath[:, 0:4, :],
                                     gath[:, 4:8, :])
                nc.vector.tensor_add(gath[:, 0:2, :], gath[:, 0:2, :],
                                     gath[:, 2:4, :])
                ST = wpool.tile([128, C_FINE], f32)
                nc.vector.tensor_add(ST[:], gath[:, 0, :], gath[:, 1, :])
                return ST

            def tail(ST):
                # S.T [128, 100] -> S [100, 128] -> P = S * (1/counts)
                ps_s = ppool.tile([C_FINE, D], ST.dtype, tag="tps")
                nc.tensor.transpose(ps_s[:], ST[:], identb_t[:])
                P_t = wpool.tile([C_FINE, D], f32)
                nc.vector.tensor_scalar_mul(P_t[:], ps_s[:], rec_t[:])

                # XT = [P.T | M.T | C.T] [128, 125] built in ONE psum bank:
                # the transpose (start=True) writes cols 0:100 and clears
                # has_written; the two prototype matmuls land on untouched
                # columns, so their first write overwrites (start=False).
                # M.T = P.T @ wm, C.T = P.T @ (wm@wc) — wmc is host-folded.
                ps_xt = ppool.tile([D, C_ALL], f32, tag="tps")
                nc.tensor.transpose(ps_xt[:, 0:C_FINE], P_t[:],
                                    ident_t[0:C_FINE, 0:C_FINE])
                nc.tensor.matmul(ps_xt[:, C_FINE:C_FINE + C_MID],
                                 P_t[:], wm_t[:], start=False, stop=False)
                nc.tensor.matmul(ps_xt[:, C_FINE + C_MID:C_ALL],
                                 P_t[:], wmc_t[:], start=False, stop=True)
                XT = wpool.tile([D, C_ALL], f32)
                nc.vector.tensor_copy(XT[:], ps_xt[:])
                XTn = wpool.tile([D, C_ALL], f32)
                nc.vector.tensor_scalar(XTn[:], ps_xt[:], -1.0, None,
                                        Alu.mult)
                x2 = wpool.tile([D, C_ALL], f32)
                nc.vector.tensor_mul(x2[:], XT[:], XT[:])
                ps_n = ppool.tile([1, C_ALL], f32, tag="tps")
                nc.tensor.matmul(ps_n[:], ones_t[:, 0:1], x2[:],
                                 start=True, stop=True)
                nm = wpool.tile([1, C_ALL], f32)
                nc.vector.tensor_scalar(nm[:], ps_n[:], 0.5, None, Alu.mult)

                def gram(a, n):
                    # psum = (n_i + n_j)/2 - G  ==  d2/2
                    sl = slice(a, a + n)
                    ps_g = ppool.tile([n, n], f32, tag="tps")
                    nc.tensor.matmul(ps_g[:], XTn[:, sl], XT[:, sl],
                                     start=True, stop=False)
                    nc.tensor.matmul(ps_g[:], ones_t[0:1, 0:n], nm[:, sl],
                                     start=False, stop=False)
                    nc.tensor.matmul(ps_g[:], nm[:, sl], ones_t[0:1, 0:n],
                                     start=False, stop=True)
                    return ps_g

                # All three Grams back-to-back on PE, then the three dist
                # sqrts in one ACT visit, straight from PSUM:
                # dist = sqrt(-2*psum + DB). The DB bias replaces the
                # max(.,0) clamp (off-diagonal d2 >= ~0.05 stays positive;
                # DB=1e-6 is ~20x above the f32 cancellation noise of the
                # diagonal). The diagonal then contributes known constants
                # to the Pearson sums, removed exactly by the fcor row.
                ps_gm = gram(C_FINE, C_MID)
                ps_gc = gram(C_FINE + C_MID, C_COARSE)
                ps_gf = gram(0, C_FINE)
                Dm_s = wpool.tile([C_MID, C_MID], f32)
                nc.scalar.activation(Dm_s[:], ps_gm[:], Act.Sqrt,
                                     bias=db_t[0:C_MID, 0:1], scale=2.0)
                Dc_s = wpool.tile([C_COARSE, C_COARSE], f32)
                nc.scalar.activation(Dc_s[:], ps_gc[:], Act.Sqrt,
                                     bias=db_t[0:C_COARSE, 0:1], scale=2.0)
                Df = wpool.tile([C_FINE, C_FINE], f32)
                nc.scalar.activation(Df[:], ps_gf[:], Act.Sqrt,
                                     bias=db_t[0:C_FINE, 0:1], scale=2.0)

                # Df-side Pearson sums overlap the tree expansion below.
                # red cols = [sTP, sT2, sP2, sT, sP]; diagonal terms are
                # O(0.01) vs sums ~1e3 — negligible, no masking needed.
                pp_s = wpool.tile([C_FINE, C_FINE], f32)
                nc.vector.reduce_sum(red[0:C_FINE, 4:5], Df[:], axis=X)
                nc.vector.scalar_tensor_tensor(
                    pp_s[:], Df[:], 1.0, Df[:], Alu.mult, Alu.mult,
                    accum_out=red[0:C_FINE, 2:3])

                # tree expansion: T = emt.T@Dm@emt + ect.T@Dc@ect
                # (Ym and Yc packed side-by-side in one psum tile)
                ps_y = ppool.tile([C_MID, 2 * C_FINE], f32, tag="tps")
                nc.tensor.matmul(ps_y[:, 0:C_FINE], Dm_s[:],
                                 emt_t[:], start=True, stop=False)
                nc.tensor.matmul(ps_y[0:C_COARSE, C_FINE:],
                                 Dc_s[:], ect_t[:],
                                 start=False, stop=True)
                YS = wpool.tile([C_MID, 2 * C_FINE], f32)
                nc.vector.tensor_copy(YS[:], ps_y[:])
                ps_T = ppool.tile([C_FINE, C_FINE], f32, tag="tps")
                nc.tensor.matmul(ps_T[:], emt_t[:], YS[:, 0:C_FINE],
                                 start=True, stop=False)
                nc.tensor.matmul(ps_T[:], ect_t[:],
                                 YS[0:C_COARSE, C_FINE:],
                                 start=False, stop=True)

                # T-side Pearson sums, back-to-back on DVE
                Tsb = wpool.tile([C_FINE, C_FINE], f32)
                nc.vector.tensor_scalar(
                    Tsb[:], ps_T[:], 1.0, 0.0, Alu.mult, Alu.add,
                    accum_out=red[0:C_FINE, 3:4])
                tp_s = wpool.tile([C_FINE, C_FINE], f32)
                nc.vector.scalar_tensor_tensor(
                    tp_s[:], Tsb[:], 1.0, Df[:], Alu.mult, Alu.mult,
                    accum_out=red[0:C_FINE, 0:1])
                tt_s = wpool.tile([C_FINE, C_FINE], f32)
                nc.vector.scalar_tensor_tensor(
                    tt_s[:], Tsb[:], 1.0, Tsb[:], Alu.mult, Alu.mult,
                    accum_out=red[0:C_FINE, 1:2])

                # reduce with a sqrt(1/(4*NPAIRS))-valued column: the
                # uniform rescale of all five sums leaves corr invariant
                # and saves the explicit 1/(4*NPAIRS) scaling ops.
                _DBG["Df"] = Df
                ps_red = ppool.tile([1, 7], f32, tag="tps")
                nc.tensor.matmul(ps_red[:, 0:5], ones_t[0:C_FINE + 1, 0:1],
                                 red[0:C_FINE + 1, 0:5],
                                 start=True, stop=False)
                nc.tensor.matmul(ps_red[:, 5:7], onesb_t[:],
                                 red[0:C_FINE + 1, 3:5],
                                 start=False, stop=True)
                f_s = wpool.tile([1, 8], f32)
                _DBG["f_s"] = f_s
                nc.vector.tensor_copy(f_s[:, 0:7], ps_red[:])

                # f_s = [F3, F4, F5, F1, F2, a*F1, a*F2]; a = 1/(4*NPAIRS)
                # [num, dt, dp] = [F3, F4, F5]/2 - [aF1*F2, aF1*F1, aF2*F2]
                # loss = 1 - num/sqrt(dt*dp + EPS)
                scr = wpool.tile([1, 3], f32)
                nc.vector.tensor_mul(scr[:, 1:3], f_s[:, 5:7], f_s[:, 3:5])
                nc.vector.tensor_mul(scr[:, 0:1], f_s[:, 5:6], f_s[:, 4:5])
                v = wpool.tile([1, 3], f32)
                nc.vector.scalar_tensor_tensor(
                    v[:], f_s[:, 0:3], 0.5, scr[:], Alu.mult,
                    Alu.subtract)
                d1 = wpool.tile([1, 1], f32)
                nc.vector.tensor_mul(d1[:], v[:, 1:2], v[:, 2:3])
                sq = wpool.tile([1, 1], f32)
                nc.scalar.activation(sq[:], d1[:], Act.Sqrt,
                                     bias=eps_t[0:1, 0:1], scale=1.0)
                rsq = wpool.tile([1, 1], f32)
                nc.vector.reciprocal(rsq[:], sq[:])
                loss_t = wpool.tile([1, 1], f32)
                nc.vector.scalar_tensor_tensor(
                    loss_t[:], v[:, 0:1], -1.0, rsq[:], Alu.mult, Alu.mult)
                last = nc.vector.tensor_scalar(loss_t[:], loss_t[:], 1.0,
                                               None, Alu.add)
                return loss_t, last

            _DBG = {"red": red}
            ST0 = all_summed()
            _DBG["ST"] = ST0
            prev, last = tail(ST0)
            for _ in range(cc_tail_reps - 1):
                # serial chain: the next AllReduce+tail cannot start before
                # the previous loss (pure dependency edge; bench only)
                if chain_cc:
                    prev, last = tail(all_summed(after=last))
                else:
                    nc.vector.tensor_copy(ST0[0:1, 0:1], prev[:])
                    prev, last = tail(ST0)
            nc.sync.dma_start(loss_d[:], prev[:])
            if debug:
                dbg_t = wpool.tile([128, 256], f32, tag="dbg")
                nc.vector.memset(dbg_t[:], 0.0)
                nc.vector.tensor_copy(dbg_t[0:C_FINE + 1, 0:8],
                                      _DBG["red"][:])
                nc.vector.tensor_copy(dbg_t[0:1, 8:16], _DBG["f_s"][:])
                nc.vector.tensor_copy(dbg_t[0:C_FINE, 16:116],
                                      _DBG["Df"][:])
                nc.vector.tensor_copy(dbg_t[0:128, 116:216],
                                      _DBG["ST"][:])
                nc.sync.dma_start(dbg_d[:], dbg_t[:])

    nc.compile()
    return nc


def _host_constants(fine2mid, fine2coarse, counts):
    f2m = np.asarray(fine2mid, dtype=np.int64)
    f2c = np.asarray(fine2coarse, dtype=np.int64)
    ident = np.eye(128, dtype=np.float32)
    ones = np.ones((128, 128), dtype=np.float32)
    rec = (1.0 / np.maximum(counts, 1.0)).astype(np.float32).reshape(
        C_FINE, 1)
    # selector / averaging matrices from the actual hierarchy inputs
    emt = (f2m[None, :] == np.arange(C_MID)[:, None]).astype(np.float32)
    cnt_m = np.maximum(np.bincount(f2m, minlength=C_MID), 1).astype(np.float32)
    wm = (emt / cnt_m[:, None]).T.astype(np.float32)     # [C_FINE, C_MID]
    # mid2coarse[m] = segment_max of fine2coarse over fines with fine2mid==m
    m2c = np.full(C_MID, -(2**31), dtype=np.int64)
    np.maximum.at(m2c, f2m, f2c)
    emc = (m2c[None, :] == np.arange(C_COARSE)[:, None]).astype(np.float32)
    cnt_c = np.maximum(emc.sum(axis=1), 1).astype(np.float32)
    wc = (emc / cnt_c[:, None]).T.astype(np.float32)     # [C_MID, C_COARSE]
    ect_sel = (f2c[None, :] == np.arange(C_COARSE)[:, None]).astype(np.float32)
    wmc = (wm @ wc).astype(np.float32)                   # [C_FINE, C_COARSE]
    # exact removal of the DB-diagonal contributions to the sums
    # cols = [sTP, sT2, sP2, sT, sP]
    r = np.sqrt(DB)
    fcor = -np.array([[2 * C_FINE * DB, 4 * C_FINE * DB, C_FINE * DB,
                       2 * C_FINE * r, C_FINE * r]], dtype=np.float64)
    onesb = np.full((C_FINE + 1, 1), 1.0 / (4.0 * NPAIRS), dtype=np.float32)
    return {
        "ident": ident, "ones": ones, "rec": rec,
        "onesb": onesb, "fcor": fcor.astype(np.float32),
        "emt": np.ascontiguousarray(emt),
        "ect": np.ascontiguousarray(ect_sel),
        "wm": np.ascontiguousarray(wm),
        "wmc": np.ascontiguousarray(wmc),
    }


def _core_layout(reps_loc, tgt_loc, offsets):
    """Sort by class, fp8-cast, chunk-tile; build the one-hot slivers.

    Returns (reps_tiled [128, N_CHUNKS*D] fp8, ohs [128, N_CHUNKS*W] fp8)
    or None if some chunk's classes fall outside its static window.
    """
    import ml_dtypes

    order = np.argsort(tgt_loc, kind="stable")
    tgt_s = tgt_loc[order]
    # chunk c holds sorted tokens [c*128, (c+1)*128); partition p = token p
    tgt_pc = tgt_s.reshape(N_CHUNKS, CHUNK).T            # [128, N_CHUNKS]
    offs = np.asarray(offsets, dtype=np.int64)[None, :]  # [1, N_CHUNKS]
    rel = tgt_pc.astype(np.int64) - offs
    if rel.min() < 0 or rel.max() >= W:
        return None
    ohs = (rel[:, :, None] == np.arange(W)[None, None, :]).astype(
        ml_dtypes.float8_e4m3).reshape(128, N_CHUNKS * W)
    reps_s = np.asarray(reps_loc)[order].astype(ml_dtypes.float8_e4m3)
    reps_tiled = np.ascontiguousarray(
        reps_s.reshape(N_CHUNKS, CHUNK, D).transpose(1, 0, 2)).reshape(
            128, N_CHUNKS * D)
    return reps_tiled, ohs


def _derive_offsets(target_fine):
    """Data-derived window starts (fallback for non-uniform targets).

    Returns offsets tuple or None: per chunk the min class over all cores,
    requiring max spread < W.
    """
    tgt = np.asarray(target_fine, dtype=np.int64)
    lo = np.full(N_CHUNKS, 2**31, dtype=np.int64)
    hi = np.full(N_CHUNKS, -1, dtype=np.int64)
    for r in range(N_CORES):
        t = np.sort(tgt[r * N_LOC:(r + 1) * N_LOC]).reshape(N_CHUNKS, CHUNK)
        lo = np.minimum(lo, t[:, 0])
        hi = np.maximum(hi, t[:, -1])
    if (hi - lo).max() >= W:
        return None
    return tuple(int(x) for x in np.clip(lo, 0, C_FINE - W))


def _make_in_maps(representations, target_fine, fine2mid, fine2coarse,
                  offsets=None):
    if offsets is None:
        offsets = _static_offsets()
    reps = np.asarray(representations, dtype=np.float32)
    tgt = np.asarray(target_fine, dtype=np.int32)
    counts = np.bincount(tgt, minlength=C_FINE).astype(np.float64)
    consts = _host_constants(fine2mid, fine2coarse, counts)
    in_maps = []
    for r in range(N_CORES):
        lo, hi = r * N_LOC, (r + 1) * N_LOC
        lay = _core_layout(reps[lo:hi], tgt[lo:hi], offsets)
        if lay is None:
            return None
        in_maps.append({"reps": lay[0], "ohs": lay[1], **consts})
    return in_maps


def kernel(representations, target_fine, fine2mid, fine2coarse):
    from concourse.bass_utils import run_bass_kernel_spmd

    assert np.asarray(representations).shape == (N, D)
    assert np.asarray(target_fine).shape == (N,)

    offsets = _static_offsets()
    in_maps = _make_in_maps(representations, target_fine,
                            fine2mid, fine2coarse, offsets)
    if in_maps is None:
        # non-uniform targets: derive windows from the data
        offsets = _derive_offsets(target_fine)
        assert offsets is not None, "class windows wider than W; raise W"
        in_maps = _make_in_maps(representations, target_fine,
                                fine2mid, fine2coarse, offsets)
        assert in_maps is not None

    if offsets not in _CACHE:
        _CACHE[offsets] = _build_program(offsets)
    nc = _CACHE[offsets]

    res = run_bass_kernel_spmd(nc, in_maps, core_ids=list(range(N_CORES)))
    loss = res.results[0]["loss"][0, 0]
    return np.asarray(loss, dtype=np.float32).reshape(())
